# revision 25
# baseline (speedup 1.0000x reference)
"""DeflateVertexToHex Trainium2 kernel (bf16 transpose-gather pipeline).

out[b, t, :] = (mean over valid s of vertex_feats[b, hex_to_vertex[t, s], :]) @ W + b

Shapes: vertex_feats [4, 20000, 512] f32, hex_to_vertex [10000, 6] i64,
W [512, 512] f32, b [512] f32 -> out [4, 10000, 512] f32.

Sharding over 8 NeuronCores: core c handles batch c//2 and hex half c%2
(5000 hexes = 39 full 128-hex tiles + one 8-hex tail tile).

The kernel is HBM-gather bound, so all gathered bytes are bf16 (rel err
~5e-3 on the real inputs, well under the 2e-2 gate):
  1. The vertex table is converted host-side to bf16 [N+1, D] with an
     appended all-zero row that padded (-1) adjacency slots remap to.
  2. SWDGE dma_gather in TRANSPOSE mode pulls each tile's 768 rows as
     2x384-idx instructions round-robined over 4 SWDGE queues ("t2",
     95.7 us per iteration measured by repeat-loop differencing, vs
     ~168 us for the previous 6x128 "t6" pipeline). Rows land as
     columns: out[p, j, i] = row_i[j*128 + p], the [D, hex*slot] layout
     the matmul wants - no PE transposes or PSUM round-trip. Queue
     number is derived from the running SWDGE instruction index mod 8
     so each tile-framework DMASW sem lane stays locked to a single
     queue (per-queue FIFO keeps each lane's cumulative completion sem
     monotone in program order).
  3. DVE pools the 6 slots with 5 bf16 adds per tile (2-byte packed
     operands run in the 2x DVE perf mode).
  4. PE accumulates out[hex, H] over 4 K-chunk bf16 matmuls straight
     from the pooled SBUF tile (lhsT = pooled[:, c, :]). When the bias
     is nonzero an extra K=1 pass adds count[hex] * b so the final
     per-hex 1/count scale distributes over it correctly.
  5. ACT applies the per-hex 1/count scale PSUM->SBUF (per-partition
     scale operand); the 128x512 result is written to DRAM as bf16 and
     upcast host-side.

HW reliability note (why kernel() spot-checks): each SWDGE gather's
descriptors fan out over the 16 DMA engines (one per idx channel; the
completion sem gets 16 x +1, one per engine, ordered behind that
engine's data writes). The tile framework's pool-WAR pre-waits keep at
most one gather in flight per DMASW lane, which makes the cumulative
lane-sem waits sound in theory; in practice a ~2%-of-runs corruption
remains where one engine's rows (p % 16 == c) of a ~2-tile window are
stale despite the sem math adding up - most often the final tiles
38-39. Structural variants (per-buffer sems via then_inc, prepare_only
+ explicit trigger, deeper pools, bigger descriptor scratch, tail
reordering) were all measured: they either slow the kernel 1.6-5x or
make the corruption worse. The chosen design keeps the fast pipeline
and catches the rare event host-side (8-column spot check against a
host reference) with a device re-run on mismatch.
"""

import numpy as np
import ml_dtypes

import concourse.bacc as bacc
import concourse.tile as tile
from concourse import mybir
from concourse.bass_utils import run_bass_kernel_spmd

F32 = mybir.dt.float32
BF16 = mybir.dt.bfloat16
I16 = mybir.dt.int16

B = 4
N = 20000
D = 512
H = 512
T = 10000
S = 6
P = 128
N_CORES = 8
T_CORE = T // 2                  # 5000 hexes per core
FULL = T_CORE // P               # 39 full 128-hex tiles
LT_H = T_CORE - FULL * P         # 8 hexes in the tail tile
TILES = FULL + 1                 # 40
PADT = TILES * P                 # 5120
GROUP = 1                        # full tiles per dma_gather instruction
                                 # (transpose-mode SWDGE fails on HW between
                                 # 768 and 1536 idx; 768 = 1 tile works)
NGROUP = FULL // GROUP           # 13
NI = GROUP * S * P               # 2304 indices per grouped gather
IDXW = S * P // 16               # 48 idx columns per full tile
TAIL_NI = 128                    # tail gather indices (48 real, s*8+h)
IDX_COLS = FULL * IDXW + TAIL_NI // 16  # 1880
JD = D // P                      # 4 K-chunks


def _patch_tile_drain():
    """This container's walrus rejects >1 sync wait on the tail InstDrain
    emitted by TileContext; split the waits across single-wait SP nops."""
    if getattr(tile.TileContext, "_drain_patch_applied", False):
        return

    def _drain_and_barrier_split(self, tick_clock, wait_clock):
        nc = self.nc
        probe = nc.sync.nop(nofuse=True)
        wait_clock.add_sem_waits(
            probe.ins, tile.ScopedClock({None: tick_clock.global_clock})
        )
        si = probe.ins.sync_info
        waits = list(si.on_wait) if si is not None else []
        if si is not None:
            si.on_wait = []
        for w in waits[1:]:
            n = nc.sync.nop(nofuse=True)
            n.ins.sync_info = mybir.SyncInfo(on_wait=[w], on_update=[])
        if waits:
            probe.ins.sync_info = mybir.SyncInfo(on_wait=[waits[0]], on_update=[])
        nc.sync.drain()
        nc.all_engine_barrier()
        assert self.sems is not None
        popped = nc._tile_sem_poison_stack.pop()
        assert popped is self._sem_poison
        nc.clear_and_free_semaphores(list(self.sems.allocated().values()))
        nc.all_engine_barrier()

    tile.TileContext._drain_and_barrier = _drain_and_barrier_split
    tile.TileContext._drain_patch_applied = True


def build_module(include_bias=True, repeat=1, nq=4, scratch=65536, gbufs=4,
                 tbufs=3, mode="t2", loop_clears=True, use_semfix=0,
                 tail_first=False):
    """mode: "t1" = transpose gather, 768 idx/tile; "t3" = transpose gather,
    3x256 idx/tile round-robined over queues; "r3" = row-mode gather 3x256 +
    DVE pair adds + PE transpose-accumulate (baseline pipeline, bf16).

    SWDGE completion-race fix (t6): each 128-idx gather's descriptors are
    split across the 16 DMA engines (one per idx channel) and each engine
    bumps the completion sem +1. The tile framework's cumulative lane-sem
    waits can therefore release a consumer while one engine still owes an
    OLD gather its share (later gathers' increments make up the count) -
    observed on HW as 8 stale rows (p%16==c) across ~2 tiles. Fix: a
    dedicated sem per (slot, pool-buffer) with threshold 16*(use+1). The
    pool WAR dependency guarantees the next gather into that buffer is not
    triggered until the previous consumer read it, so the per-buffer count
    is unpollutable."""
    _patch_tile_drain()
    nc = bacc.Bacc(
        "TRN2",
        target_bir_lowering=False,
        debug=False,
        num_swdge_queues=nq,
        dynamic_dma_scratch_size=scratch,
    )
    vtx = nc.declare_dram_parameter("vtx", [N + 1, D], BF16, isOutput=False)
    wm = nc.declare_dram_parameter("wmat", [D, H], BF16, isOutput=False)
    brow = nc.declare_dram_parameter("brow", [1, H], BF16, isOutput=False)
    cnt = nc.declare_dram_parameter("cnt", [1, PADT], BF16, isOutput=False)
    idx = nc.declare_dram_parameter("idx", [P, IDX_COLS], I16, isOutput=False)
    invc = nc.declare_dram_parameter("invc", [P, TILES], F32, isOutput=False)
    out = nc.declare_dram_parameter("out", [PADT, H], BF16, isOutput=True)

    # Per-(slot, buf) DMA-completion sems + one for the tail gather. Each
    # sem must be touched by exactly ONE SWDGE queue; gbufs % 4 == 0 makes
    # the queue of (slot s, buf k) constant: sw_i = 6t+s with t = k+gbufs*m
    # gives lane (6k+s) % 8 for every use (6*gbufs % 8 == 0).
    if mode == "t6" and use_semfix:
        assert gbufs % 4 == 0, "t6 per-buffer sems need gbufs % 4 == 0"
    q_of = lambda s, k: ((6 * k + s) % 8) % nq
    gsems = [[nc.alloc_semaphore(f"gs{s}b{k}") for k in range(gbufs)]
             for s in range(S)]
    tailsem = nc.alloc_semaphore("gstail")
    q_tail = (FULL * S % 8) % nq

    def _clear_gsems():
        for s in range(S):
            for k in range(gbufs):
                nc.gpsimd.inc_swdge_sem(
                    [gsems[s][k]], [0], queue_num=q_of(s, k), mode="wr"
                )
        nc.gpsimd.inc_swdge_sem([tailsem], [0], queue_num=q_tail, mode="wr")

    if use_semfix >= 2:
        _clear_gsems()
        nc.all_engine_barrier()

    with tile.TileContext(nc) as tc:
        with (
            tc.tile_pool(name="const", bufs=1) as constp,
            tc.tile_pool(name="gather", bufs=gbufs) as gpool,
            tc.tile_pool(name="tmp", bufs=tbufs) as tmp,
            tc.tile_pool(name="pl", bufs=3) as plp,
            tc.tile_pool(name="osb", bufs=4) as osb,
            tc.tile_pool(name="mmps", bufs=3, space="PSUM") as mmps,
            tc.tile_pool(name="ptps", bufs=2, space="PSUM") as ptps,
        ):
            # idx first: group 0's slice in its own small DMA so gather 0
            # isn't held behind the full index load.
            idx_sb = constp.tile([P, IDX_COLS], I16)
            c0 = GROUP * IDXW
            nc.sync.dma_start(idx_sb[:, :c0], idx[:, :c0])
            nc.sync.dma_start(idx_sb[:, c0:], idx[:, c0:])
            w_sb = constp.tile([P, JD * H], BF16)
            for c in range(JD):
                nc.sync.dma_start(
                    w_sb[:, c * H : (c + 1) * H], wm[c * P : (c + 1) * P, :]
                )
            invc_sb = constp.tile([P, TILES], F32)
            nc.sync.dma_start(invc_sb[:], invc[:])
            b_sb = constp.tile([1, H], BF16)
            nc.sync.dma_start(b_sb[:], brow[:])
            cnt_sb = constp.tile([1, PADT], BF16)
            nc.sync.dma_start(cnt_sb[:], cnt[:])
            ident = None
            if mode == "r3":
                from concourse.masks import make_identity

                ident = constp.tile([P, P], BF16)
                make_identity(nc, ident[:])

            import contextlib

            sw_i = [0]          # running SWDGE DMA index: sem lane = i % 8,
                                # so queue must be a function of i % 8

            def _swq():
                q = (sw_i[0] % 8) % nq
                sw_i[0] += 1
                return q

            loop_ctx = tc.For_i(0, repeat, 1) if repeat > 1 else contextlib.nullcontext()
            with loop_ctx:
                if repeat > 1 and loop_clears and use_semfix >= 2:
                    # reset the manual gather sems each iteration so the
                    # waits below keep their static thresholds meaningful
                    _clear_gsems()
                tile_order = ([TILES - 1] + list(range(FULL))) if tail_first else range(TILES)
                for t in tile_order:
                    is_tail = t == TILES - 1
                    hexes = LT_H if is_tail else P
                    col = t * IDXW

                    def _finish(pl_lhsT_chunks):
                        """pl_lhsT_chunks: c -> AP [128(K), hexes] bf16."""
                        mmp = mmps.tile([P, H], F32)
                        for c in range(JD):
                            nc.tensor.matmul(
                                mmp[:hexes, :],
                                lhsT=pl_lhsT_chunks(c),
                                rhs=w_sb[:, c * H : (c + 1) * H],
                                start=(c == 0),
                                stop=(c == JD - 1 and not include_bias),
                            )
                        if include_bias:
                            nc.tensor.matmul(
                                mmp[:hexes, :],
                                lhsT=cnt_sb[0:1, t * P : t * P + hexes],
                                rhs=b_sb[0:1, :],
                                start=False,
                                stop=True,
                            )
                        o = osb.tile([P, H], BF16, tag="o")
                        nc.scalar.mul(
                            o[:hexes, :], mmp[:hexes, :], invc_sb[:hexes, t : t + 1]
                        )
                        nc.sync.dma_start(
                            out[t * P : t * P + hexes, :], o[:hexes, :]
                        )

                    if is_tail or mode == "t1":
                        ni = TAIL_NI if is_tail else NI
                        gt = gpool.tile(
                            [P, JD, ni], BF16, tag="gtail" if is_tail else "g"
                        )
                        tq = _swq()
                        if is_tail and use_semfix == 3:
                            nc.gpsimd.dma_gather(
                                gt[:],
                                vtx[:],
                                idx_sb[:, col : col + ni // 16],
                                ni,
                                ni,
                                D,
                                transpose=True,
                                prepare_only=True,
                                sem=tailsem,
                                queue_num=tq,
                            )
                            nc.gpsimd.trigger_dma(count=None, queue_num=tq)
                            nc.vector.wait_ge(tailsem, 16)
                        else:
                            gi = nc.gpsimd.dma_gather(
                                gt[:],
                                vtx[:],
                                idx_sb[:, col : col + ni // 16],
                                ni,
                                ni,
                                D,
                                transpose=True,
                                queue_num=tq,
                            )
                            if is_tail and use_semfix:
                                gi.then_inc(tailsem, 16)
                                nc.vector.wait_ge(tailsem, 16)
                        w3 = 3 * hexes          # three-slot block width
                        # slot pool: (s, s+3) pairs then fold 3 -> 1
                        q = tmp.tile([P, JD, w3], BF16, tag="q")
                        nc.vector.tensor_add(
                            q[:], gt[:, :, 0:w3], gt[:, :, w3 : 2 * w3]
                        )
                        r = tmp.tile([P, JD, hexes], BF16, tag="r")
                        nc.vector.tensor_add(
                            r[:], q[:, :, 0:hexes], q[:, :, hexes : 2 * hexes]
                        )
                        pl = plp.tile([P, JD, hexes], BF16, tag="p")
                        nc.vector.tensor_add(pl[:], r[:], q[:, :, 2 * hexes : w3])
                        _finish(lambda c: pl[:, c, :])
                    elif mode == "t2":
                        # 2 gathers x 384 idx (3 slots each). With gbufs<=4
                        # at most 2*gbufs<=8 gathers are in flight, so two
                        # gathers 8 apart (same DMASW lane) are never
                        # concurrently outstanding: the framework's
                        # cumulative lane-sem waits are race-free without
                        # any extra semaphores (see use_semfix note above).
                        halves = []
                        for hg in range(2):
                            gp = gpool.tile([P, JD, 3 * P], BF16, tag=f"h{hg}")
                            nc.gpsimd.dma_gather(
                                gp[:],
                                vtx[:],
                                idx_sb[:, col + hg * 24 : col + (hg + 1) * 24],
                                3 * P,
                                3 * P,
                                D,
                                transpose=True,
                                queue_num=_swq(),
                            )
                            halves.append(gp)
                        g0, g1 = halves
                        a1 = tmp.tile([P, JD, P], BF16, tag="q0")
                        nc.vector.tensor_add(
                            a1[:], g0[:, :, 0:P], g0[:, :, P : 2 * P]
                        )
                        a2 = tmp.tile([P, JD, P], BF16, tag="q1")
                        nc.vector.tensor_add(
                            a2[:], g0[:, :, 2 * P : 3 * P], g1[:, :, 0:P]
                        )
                        a3 = tmp.tile([P, JD, P], BF16, tag="q2")
                        nc.vector.tensor_add(
                            a3[:], g1[:, :, P : 2 * P], g1[:, :, 2 * P : 3 * P]
                        )
                        r = tmp.tile([P, JD, P], BF16, tag="r")
                        nc.vector.tensor_add(r[:], a1[:], a2[:])
                        pl = plp.tile([P, JD, P], BF16, tag="p")
                        nc.vector.tensor_add(pl[:], r[:], a3[:])
                        _finish(lambda c: pl[:, c, :])
                    elif mode == "t6":
                        buf = t % gbufs
                        thr = 16 * (t // gbufs + 1)
                        gps = []
                        queues_used = set()
                        for s in range(6):
                            gp = gpool.tile([P, JD, P], BF16, tag=f"s{s}")
                            q = _swq()
                            if use_semfix == 3:
                                nc.gpsimd.dma_gather(
                                    gp[:],
                                    vtx[:],
                                    idx_sb[:, col + s * 8 : col + (s + 1) * 8],
                                    P,
                                    P,
                                    D,
                                    transpose=True,
                                    prepare_only=True,
                                    sem=gsems[s][buf],
                                    queue_num=q,
                                )
                                queues_used.add(q)
                            else:
                                gi = nc.gpsimd.dma_gather(
                                    gp[:],
                                    vtx[:],
                                    idx_sb[:, col + s * 8 : col + (s + 1) * 8],
                                    P,
                                    P,
                                    D,
                                    transpose=True,
                                    queue_num=q,
                                )
                                if use_semfix:
                                    gi.then_inc(gsems[s][buf], 16)
                            gps.append(gp)
                        for q in sorted(queues_used):
                            nc.gpsimd.trigger_dma(count=None, queue_num=q)
                        q01 = tmp.tile([P, JD, P], BF16, tag="q0")
                        if use_semfix >= 2:
                            nc.vector.wait_ge(gsems[0][buf], thr)
                            nc.vector.wait_ge(gsems[1][buf], thr)
                        nc.vector.tensor_add(q01[:], gps[0][:], gps[1][:])
                        q23 = tmp.tile([P, JD, P], BF16, tag="q1")
                        if use_semfix >= 2:
                            nc.vector.wait_ge(gsems[2][buf], thr)
                            nc.vector.wait_ge(gsems[3][buf], thr)
                        nc.vector.tensor_add(q23[:], gps[2][:], gps[3][:])
                        q45 = tmp.tile([P, JD, P], BF16, tag="q2")
                        if use_semfix >= 2:
                            nc.vector.wait_ge(gsems[4][buf], thr)
                            nc.vector.wait_ge(gsems[5][buf], thr)
                        nc.vector.tensor_add(q45[:], gps[4][:], gps[5][:])
                        r = tmp.tile([P, JD, P], BF16, tag="r")
                        nc.vector.tensor_add(r[:], q01[:], q23[:])
                        pl = plp.tile([P, JD, P], BF16, tag="p")
                        nc.vector.tensor_add(pl[:], r[:], q45[:])
                        _finish(lambda c: pl[:, c, :])
                    elif mode == "t3":
                        parts = []
                        for pi in range(3):
                            gp = gpool.tile([P, JD, 2 * P], BF16, tag=f"g{pi}")
                            nc.gpsimd.dma_gather(
                                gp[:],
                                vtx[:],
                                idx_sb[:, col + pi * 16 : col + (pi + 1) * 16],
                                2 * P,
                                2 * P,
                                D,
                                transpose=True,
                                queue_num=_swq(),
                            )
                            sp = tmp.tile([P, JD, P], BF16, tag=f"q{pi}")
                            nc.vector.tensor_add(
                                sp[:], gp[:, :, 0:P], gp[:, :, P : 2 * P]
                            )
                            parts.append(sp)
                        r = tmp.tile([P, JD, P], BF16, tag="r")
                        nc.vector.tensor_add(r[:], parts[0][:], parts[1][:])
                        pl = plp.tile([P, JD, P], BF16, tag="p")
                        nc.vector.tensor_add(pl[:], r[:], parts[2][:])
                        _finish(lambda c: pl[:, c, :])
                    elif mode == "r3":
                        parts = []
                        for pi in range(3):
                            gp = gpool.tile([P, 2, D], BF16, tag=f"g{pi}")
                            nc.gpsimd.dma_gather(
                                gp[:],
                                vtx[:],
                                idx_sb[:, col + pi * 16 : col + (pi + 1) * 16],
                                2 * P,
                                2 * P,
                                D,
                                queue_num=_swq(),
                            )
                            sp = tmp.tile([P, D], BF16, tag=f"q{pi}")
                            nc.vector.tensor_add(sp[:], gp[:, 0, :], gp[:, 1, :])
                            parts.append(sp)
                        ptp = ptps.tile([P, D], BF16)
                        for c in range(JD):
                            for pi in range(3):
                                nc.tensor.matmul(
                                    ptp[:, c * P : (c + 1) * P],
                                    lhsT=parts[pi][:, c * P : (c + 1) * P],
                                    rhs=ident[:],
                                    is_transpose=True,
                                    start=(pi == 0),
                                    stop=(pi == 2),
                                )
                        pl = plp.tile([P, D], BF16, tag="p")
                        nc.scalar.copy(pl[:], ptp[:])
                        _finish(lambda c: pl[:, c * P : (c + 1) * P])
                    else:
                        raise ValueError(mode)
    nc.finalize()
    return nc


def prep_inputs(vertex_feats, hex_to_vertex, W, b):
    """Host-side prep -> per-core in_maps."""
    vertex_feats = np.asarray(vertex_feats)
    hex_to_vertex = np.asarray(hex_to_vertex)
    W16 = np.ascontiguousarray(np.asarray(W).astype(ml_dtypes.bfloat16))
    b16 = np.asarray(b).astype(ml_dtypes.bfloat16).reshape(1, H)

    mask = hex_to_vertex >= 0
    safe = np.where(mask, hex_to_vertex, N).astype(np.int16)       # [T, 6]
    cntc = np.maximum(mask.sum(axis=1), 1).astype(np.float32)      # [T]
    inv = (1.0 / cntc).astype(np.float32)

    vtx_pads = []
    for bi in range(B):
        vp = np.zeros((N + 1, D), dtype=ml_dtypes.bfloat16)
        vp[:N] = vertex_feats[bi].astype(ml_dtypes.bfloat16)
        vtx_pads.append(vp)

    half_arrays = []
    for hh in range(2):
        sl = slice(hh * T_CORE, (hh + 1) * T_CORE)
        safe_h = safe[sl]                                          # [5000, 6]
        cnt_pad = np.ones(PADT, dtype=np.float32)
        cnt_pad[:T_CORE] = cntc[sl]
        inv_pad = np.ones(PADT, dtype=np.float32)
        inv_pad[:T_CORE] = inv[sl]
        # full tiles: flat[t, s*128 + p] = safe_h[t*128 + p, s]
        flat_full = (
            safe_h[: FULL * P]
            .reshape(FULL, P, S)
            .transpose(0, 2, 1)
            .reshape(FULL * S * P)
        )
        # tail: i = s*8 + h for s < 6, h < 8; rest -> zero row N
        flat_tail = np.full(TAIL_NI, N, dtype=np.int16)
        flat_tail[: S * LT_H] = safe_h[FULL * P : FULL * P + LT_H].T.reshape(
            S * LT_H
        )
        flat = np.concatenate([flat_full, flat_tail])
        # SWDGE idx wrap: column j, row p16 = flat[j*16 + p16]
        idx16 = flat.reshape(IDX_COLS, 16).T.astype(np.int16)
        idx_full = np.ascontiguousarray(np.tile(idx16, (8, 1)))    # 8 Q7 groups
        invc_arr = np.ascontiguousarray(inv_pad.reshape(TILES, P).T)  # [P, TILES]
        cnt_row = np.ascontiguousarray(
            cnt_pad.reshape(1, PADT).astype(ml_dtypes.bfloat16)
        )
        half_arrays.append((idx_full, invc_arr, cnt_row))

    in_maps = []
    for c in range(N_CORES):
        bi, hh = c // 2, c % 2
        idx_full, invc_arr, cnt_row = half_arrays[hh]
        in_maps.append(
            {
                "vtx": vtx_pads[bi],
                "wmat": W16,
                "brow": b16,
                "cnt": cnt_row,
                "idx": idx_full,
                "invc": invc_arr,
            }
        )
    return in_maps


def assemble_output(results):
    out = np.empty((B, T, H), dtype=np.float32)
    for c in range(N_CORES):
        bi, hh = c // 2, c % 2
        out[bi, hh * T_CORE : (hh + 1) * T_CORE] = (
            np.asarray(results[c]["out"][:T_CORE]).astype(np.float32)
        )
    return out


_CACHE = {}

# Host-side spot check: SWDGE gathers have a rare (~2%/run) silent-corruption
# mode on this HW where one DMA engine's rows of a ~2-tile window are stale
# (see the race notes in build_module). The kernel verifies 8 of the 512
# output columns against a host reference and re-runs the device program on
# mismatch. bf16 noise per element stays under ~0.03; corruption shows as
# O(0.5) errors on >=16 rows, so 8 columns at threshold 0.12 make a miss
# astronomically unlikely while false positives are ~impossible.
_CHECK_COLS = np.arange(0, H, H // 8)
_CHECK_THRESH = 0.12


def _spotcheck_ref(vertex_feats, hex_to_vertex, W, b):
    vf16 = np.asarray(vertex_feats).astype(ml_dtypes.bfloat16).astype(np.float32)
    h2v = np.asarray(hex_to_vertex)
    mask = h2v >= 0
    safe = np.where(mask, h2v, 0)
    cnt = np.maximum(mask.sum(1), 1).astype(np.float32)
    W8 = np.asarray(W)[:, _CHECK_COLS].astype(np.float32)
    b8 = np.asarray(b)[_CHECK_COLS].astype(np.float32)
    ref = np.empty((B, T, len(_CHECK_COLS)), np.float32)
    m = mask[:, :, None].astype(np.float32)
    for bi in range(B):
        pooled = (vf16[bi][safe] * m).sum(1) / cnt[:, None]     # (T, D)
        ref[bi] = pooled @ W8 + b8
    return ref


def kernel(vertex_feats, hex_to_vertex, W, b):
    include_bias = bool(np.any(np.asarray(b)))
    nc = _CACHE.get(include_bias)
    if nc is None:
        nc = build_module(include_bias=include_bias)
        _CACHE[include_bias] = nc
        _CACHE["nc"] = nc
    in_maps = prep_inputs(vertex_feats, hex_to_vertex, W, b)
    ref8 = _spotcheck_ref(vertex_feats, hex_to_vertex, W, b)
    out = None
    for attempt in range(6):
        res = run_bass_kernel_spmd(nc, in_maps, list(range(N_CORES)))
        out = assemble_output(res.results)
        bad = np.abs(out[:, :, _CHECK_COLS] - ref8) > _CHECK_THRESH
        if not bad.any():
            break
        nbad = int(bad.any(axis=2).sum())
        print(f"kernel: spot-check failed on {nbad} rows "
              f"(attempt {attempt}), re-running")
    return out


if __name__ == "__main__":
    rng = np.random.default_rng(0)
    vf = rng.standard_normal((B, N, D), dtype=np.float32)
    h2v = rng.integers(-1, N, size=(T, S), dtype=np.int64)
    W = (rng.standard_normal((D, H)) / np.sqrt(D)).astype(np.float32)
    b = np.zeros(H, dtype=np.float32)
    out = kernel(vertex_feats=vf, hex_to_vertex=h2v, W=W, b=b)
    print("out", out.shape, out.dtype, float(np.abs(out).max()))



# revision 26
# speedup vs baseline: 1.0017x; 1.0017x over previous
"""DeflateVertexToHex Trainium2 kernel (bf16 transpose-gather pipeline).

out[b, t, :] = (mean over valid s of vertex_feats[b, hex_to_vertex[t, s], :]) @ W + b

Shapes: vertex_feats [4, 20000, 512] f32, hex_to_vertex [10000, 6] i64,
W [512, 512] f32, b [512] f32 -> out [4, 10000, 512] f32.

Sharding over 8 NeuronCores: core c handles batch c//2 and hex half c%2
(5000 hexes = 39 full 128-hex tiles + one 8-hex tail tile).

The kernel is HBM-gather bound, so all gathered bytes are bf16 (rel err
~5e-3 on the real inputs, well under the 2e-2 gate):
  1. The vertex table is converted host-side to bf16 [N+1, D] with an
     appended all-zero row that padded (-1) adjacency slots remap to.
  2. SWDGE dma_gather in TRANSPOSE mode pulls each tile's 768 rows as
     2x384-idx instructions round-robined over 4 SWDGE queues ("t2",
     95.7 us per iteration measured by repeat-loop differencing, vs
     ~168 us for the previous 6x128 "t6" pipeline). Rows land as
     columns: out[p, j, i] = row_i[j*128 + p], the [D, hex*slot] layout
     the matmul wants - no PE transposes or PSUM round-trip. Queue
     number is derived from the running SWDGE instruction index mod 8
     so each tile-framework DMASW sem lane stays locked to a single
     queue (per-queue FIFO keeps each lane's cumulative completion sem
     monotone in program order).
  3. DVE pools the 6 slots with 5 bf16 adds per tile (2-byte packed
     operands run in the 2x DVE perf mode).
  4. PE accumulates out[hex, H] over 4 K-chunk bf16 matmuls straight
     from the pooled SBUF tile (lhsT = pooled[:, c, :]). When the bias
     is nonzero an extra K=1 pass adds count[hex] * b so the final
     per-hex 1/count scale distributes over it correctly.
  5. ACT applies the per-hex 1/count scale PSUM->SBUF (per-partition
     scale operand); the 128x512 result is written to DRAM as bf16 and
     upcast host-side.

HW reliability note (why kernel() spot-checks): each SWDGE gather's
descriptors fan out over the 16 DMA engines (one per idx channel; the
completion sem gets 16 x +1, one per engine, ordered behind that
engine's data writes). The tile framework's pool-WAR pre-waits keep at
most one gather in flight per DMASW lane, which makes the cumulative
lane-sem waits sound in theory; in practice a ~2%-of-runs corruption
remains where one engine's rows (p % 16 == c) of a ~2-tile window are
stale despite the sem math adding up - most often the final tiles
38-39. Structural variants (per-buffer sems via then_inc, prepare_only
+ explicit trigger, deeper pools, bigger descriptor scratch, tail
reordering) were all measured: they either slow the kernel 1.6-5x or
make the corruption worse. The chosen design keeps the fast pipeline
and catches the rare event host-side (8-column spot check against a
host reference) with a device re-run on mismatch.
"""

import numpy as np
import ml_dtypes

import concourse.bacc as bacc
import concourse.tile as tile
from concourse import mybir
from concourse.bass_utils import run_bass_kernel_spmd

F32 = mybir.dt.float32
BF16 = mybir.dt.bfloat16
I16 = mybir.dt.int16

B = 4
N = 20000
D = 512
H = 512
T = 10000
S = 6
P = 128
N_CORES = 8
T_CORE = T // 2                  # 5000 hexes per core
FULL = T_CORE // P               # 39 full 128-hex tiles
LT_H = T_CORE - FULL * P         # 8 hexes in the tail tile
TILES = FULL + 1                 # 40
PADT = TILES * P                 # 5120
GROUP = 1                        # full tiles per dma_gather instruction
                                 # (transpose-mode SWDGE fails on HW between
                                 # 768 and 1536 idx; 768 = 1 tile works)
NGROUP = FULL // GROUP           # 13
NI = GROUP * S * P               # 2304 indices per grouped gather
IDXW = S * P // 16               # 48 idx columns per full tile
TAIL_NI = 128                    # tail gather indices (48 real, s*8+h)
IDX_COLS = FULL * IDXW + TAIL_NI // 16  # 1880
JD = D // P                      # 4 K-chunks


def _patch_tile_drain():
    """This container's walrus rejects >1 sync wait on the tail InstDrain
    emitted by TileContext; split the waits across single-wait SP nops."""
    if getattr(tile.TileContext, "_drain_patch_applied", False):
        return

    def _drain_and_barrier_split(self, tick_clock, wait_clock):
        nc = self.nc
        probe = nc.sync.nop(nofuse=True)
        wait_clock.add_sem_waits(
            probe.ins, tile.ScopedClock({None: tick_clock.global_clock})
        )
        si = probe.ins.sync_info
        waits = list(si.on_wait) if si is not None else []
        if si is not None:
            si.on_wait = []
        for w in waits[1:]:
            n = nc.sync.nop(nofuse=True)
            n.ins.sync_info = mybir.SyncInfo(on_wait=[w], on_update=[])
        if waits:
            probe.ins.sync_info = mybir.SyncInfo(on_wait=[waits[0]], on_update=[])
        nc.sync.drain()
        nc.all_engine_barrier()
        assert self.sems is not None
        popped = nc._tile_sem_poison_stack.pop()
        assert popped is self._sem_poison
        nc.clear_and_free_semaphores(list(self.sems.allocated().values()))
        nc.all_engine_barrier()

    tile.TileContext._drain_and_barrier = _drain_and_barrier_split
    tile.TileContext._drain_patch_applied = True


def build_module(include_bias=True, repeat=1, nq=4, scratch=65536, gbufs=4,
                 tbufs=4, mode="t2", loop_clears=True, use_semfix=0,
                 tail_first=False):
    """mode: "t1" = transpose gather, 768 idx/tile; "t3" = transpose gather,
    3x256 idx/tile round-robined over queues; "r3" = row-mode gather 3x256 +
    DVE pair adds + PE transpose-accumulate (baseline pipeline, bf16).

    SWDGE completion-race fix (t6): each 128-idx gather's descriptors are
    split across the 16 DMA engines (one per idx channel) and each engine
    bumps the completion sem +1. The tile framework's cumulative lane-sem
    waits can therefore release a consumer while one engine still owes an
    OLD gather its share (later gathers' increments make up the count) -
    observed on HW as 8 stale rows (p%16==c) across ~2 tiles. Fix: a
    dedicated sem per (slot, pool-buffer) with threshold 16*(use+1). The
    pool WAR dependency guarantees the next gather into that buffer is not
    triggered until the previous consumer read it, so the per-buffer count
    is unpollutable."""
    _patch_tile_drain()
    nc = bacc.Bacc(
        "TRN2",
        target_bir_lowering=False,
        debug=False,
        num_swdge_queues=nq,
        dynamic_dma_scratch_size=scratch,
    )
    vtx = nc.declare_dram_parameter("vtx", [N + 1, D], BF16, isOutput=False)
    wm = nc.declare_dram_parameter("wmat", [D, H], BF16, isOutput=False)
    brow = nc.declare_dram_parameter("brow", [1, H], BF16, isOutput=False)
    cnt = nc.declare_dram_parameter("cnt", [1, PADT], BF16, isOutput=False)
    idx = nc.declare_dram_parameter("idx", [P, IDX_COLS], I16, isOutput=False)
    invc = nc.declare_dram_parameter("invc", [P, TILES], F32, isOutput=False)
    out = nc.declare_dram_parameter("out", [PADT, H], BF16, isOutput=True)

    # Per-(slot, buf) DMA-completion sems + one for the tail gather. Each
    # sem must be touched by exactly ONE SWDGE queue; gbufs % 4 == 0 makes
    # the queue of (slot s, buf k) constant: sw_i = 6t+s with t = k+gbufs*m
    # gives lane (6k+s) % 8 for every use (6*gbufs % 8 == 0).
    if mode == "t6" and use_semfix:
        assert gbufs % 4 == 0, "t6 per-buffer sems need gbufs % 4 == 0"
    q_of = lambda s, k: ((6 * k + s) % 8) % nq
    gsems = [[nc.alloc_semaphore(f"gs{s}b{k}") for k in range(gbufs)]
             for s in range(S)]
    tailsem = nc.alloc_semaphore("gstail")
    q_tail = (FULL * S % 8) % nq

    def _clear_gsems():
        for s in range(S):
            for k in range(gbufs):
                nc.gpsimd.inc_swdge_sem(
                    [gsems[s][k]], [0], queue_num=q_of(s, k), mode="wr"
                )
        nc.gpsimd.inc_swdge_sem([tailsem], [0], queue_num=q_tail, mode="wr")

    if use_semfix >= 2:
        _clear_gsems()
        nc.all_engine_barrier()

    import contextlib

    with tile.TileContext(nc) as tc:
        with (
            tc.tile_pool(name="const", bufs=1) as constp,
            tc.tile_pool(name="gather", bufs=gbufs) as gpool,
            tc.tile_pool(name="tmp", bufs=tbufs) as tmp,
            tc.tile_pool(name="pl", bufs=4) as plp,
            tc.tile_pool(name="osb", bufs=6) as osb,
            tc.tile_pool(name="mmps", bufs=4, space="PSUM") as mmps,
            tc.tile_pool(name="ptps", bufs=2, space="PSUM")
            if mode == "r3" else contextlib.nullcontext() as ptps,
        ):
            # idx first: group 0's slice in its own small DMA so gather 0
            # isn't held behind the full index load.
            idx_sb = constp.tile([P, IDX_COLS], I16)
            c0 = GROUP * IDXW
            nc.sync.dma_start(idx_sb[:, :c0], idx[:, :c0])
            nc.sync.dma_start(idx_sb[:, c0:], idx[:, c0:])
            w_sb = constp.tile([P, JD * H], BF16)
            for c in range(JD):
                nc.sync.dma_start(
                    w_sb[:, c * H : (c + 1) * H], wm[c * P : (c + 1) * P, :]
                )
            invc_sb = constp.tile([P, TILES], F32)
            nc.sync.dma_start(invc_sb[:], invc[:])
            b_sb = constp.tile([1, H], BF16)
            nc.sync.dma_start(b_sb[:], brow[:])
            cnt_sb = constp.tile([1, PADT], BF16)
            nc.sync.dma_start(cnt_sb[:], cnt[:])
            ident = None
            if mode == "r3":
                from concourse.masks import make_identity

                ident = constp.tile([P, P], BF16)
                make_identity(nc, ident[:])

            import contextlib

            sw_i = [0]          # running SWDGE DMA index: sem lane = i % 8,
                                # so queue must be a function of i % 8

            def _swq():
                q = (sw_i[0] % 8) % nq
                sw_i[0] += 1
                return q

            loop_ctx = tc.For_i(0, repeat, 1) if repeat > 1 else contextlib.nullcontext()
            with loop_ctx:
                if repeat > 1 and loop_clears and use_semfix >= 2:
                    # reset the manual gather sems each iteration so the
                    # waits below keep their static thresholds meaningful
                    _clear_gsems()
                tile_order = ([TILES - 1] + list(range(FULL))) if tail_first else range(TILES)
                for t in tile_order:
                    is_tail = t == TILES - 1
                    hexes = LT_H if is_tail else P
                    col = t * IDXW

                    def _finish(pl_lhsT_chunks):
                        """pl_lhsT_chunks: c -> AP [128(K), hexes] bf16."""
                        mmp = mmps.tile([P, H], F32)
                        for c in range(JD):
                            nc.tensor.matmul(
                                mmp[:hexes, :],
                                lhsT=pl_lhsT_chunks(c),
                                rhs=w_sb[:, c * H : (c + 1) * H],
                                start=(c == 0),
                                stop=(c == JD - 1 and not include_bias),
                            )
                        if include_bias:
                            nc.tensor.matmul(
                                mmp[:hexes, :],
                                lhsT=cnt_sb[0:1, t * P : t * P + hexes],
                                rhs=b_sb[0:1, :],
                                start=False,
                                stop=True,
                            )
                        o = osb.tile([P, H], BF16, tag="o")
                        nc.scalar.mul(
                            o[:hexes, :], mmp[:hexes, :], invc_sb[:hexes, t : t + 1]
                        )
                        nc.sync.dma_start(
                            out[t * P : t * P + hexes, :], o[:hexes, :]
                        )

                    if is_tail or mode == "t1":
                        ni = TAIL_NI if is_tail else NI
                        gt = gpool.tile(
                            [P, JD, ni], BF16, tag="gtail" if is_tail else "g"
                        )
                        tq = _swq()
                        if is_tail and use_semfix == 3:
                            nc.gpsimd.dma_gather(
                                gt[:],
                                vtx[:],
                                idx_sb[:, col : col + ni // 16],
                                ni,
                                ni,
                                D,
                                transpose=True,
                                prepare_only=True,
                                sem=tailsem,
                                queue_num=tq,
                            )
                            nc.gpsimd.trigger_dma(count=None, queue_num=tq)
                            nc.vector.wait_ge(tailsem, 16)
                        else:
                            gi = nc.gpsimd.dma_gather(
                                gt[:],
                                vtx[:],
                                idx_sb[:, col : col + ni // 16],
                                ni,
                                ni,
                                D,
                                transpose=True,
                                queue_num=tq,
                            )
                            if is_tail and use_semfix:
                                gi.then_inc(tailsem, 16)
                                nc.vector.wait_ge(tailsem, 16)
                        w3 = 3 * hexes          # three-slot block width
                        # slot pool: (s, s+3) pairs then fold 3 -> 1
                        q = tmp.tile([P, JD, w3], BF16, tag="q")
                        nc.vector.tensor_add(
                            q[:], gt[:, :, 0:w3], gt[:, :, w3 : 2 * w3]
                        )
                        r = tmp.tile([P, JD, hexes], BF16, tag="r")
                        nc.vector.tensor_add(
                            r[:], q[:, :, 0:hexes], q[:, :, hexes : 2 * hexes]
                        )
                        pl = plp.tile([P, JD, hexes], BF16, tag="p")
                        nc.vector.tensor_add(pl[:], r[:], q[:, :, 2 * hexes : w3])
                        _finish(lambda c: pl[:, c, :])
                    elif mode == "t2":
                        # 2 gathers x 384 idx (3 slots each). With gbufs<=4
                        # at most 2*gbufs<=8 gathers are in flight, so two
                        # gathers 8 apart (same DMASW lane) are never
                        # concurrently outstanding: the framework's
                        # cumulative lane-sem waits are race-free without
                        # any extra semaphores (see use_semfix note above).
                        halves = []
                        for hg in range(2):
                            gp = gpool.tile([P, JD, 3 * P], BF16, tag=f"h{hg}")
                            nc.gpsimd.dma_gather(
                                gp[:],
                                vtx[:],
                                idx_sb[:, col + hg * 24 : col + (hg + 1) * 24],
                                3 * P,
                                3 * P,
                                D,
                                transpose=True,
                                queue_num=_swq(),
                            )
                            halves.append(gp)
                        g0, g1 = halves
                        a1 = tmp.tile([P, JD, P], BF16, tag="q0")
                        nc.vector.tensor_add(
                            a1[:], g0[:, :, 0:P], g0[:, :, P : 2 * P]
                        )
                        a2 = tmp.tile([P, JD, P], BF16, tag="q1")
                        nc.vector.tensor_add(
                            a2[:], g0[:, :, 2 * P : 3 * P], g1[:, :, 0:P]
                        )
                        a3 = tmp.tile([P, JD, P], BF16, tag="q2")
                        nc.vector.tensor_add(
                            a3[:], g1[:, :, P : 2 * P], g1[:, :, 2 * P : 3 * P]
                        )
                        r = tmp.tile([P, JD, P], BF16, tag="r")
                        nc.vector.tensor_add(r[:], a1[:], a2[:])
                        pl = plp.tile([P, JD, P], BF16, tag="p")
                        nc.vector.tensor_add(pl[:], r[:], a3[:])
                        _finish(lambda c: pl[:, c, :])
                    elif mode == "t6":
                        buf = t % gbufs
                        thr = 16 * (t // gbufs + 1)
                        gps = []
                        queues_used = set()
                        for s in range(6):
                            gp = gpool.tile([P, JD, P], BF16, tag=f"s{s}")
                            q = _swq()
                            if use_semfix == 3:
                                nc.gpsimd.dma_gather(
                                    gp[:],
                                    vtx[:],
                                    idx_sb[:, col + s * 8 : col + (s + 1) * 8],
                                    P,
                                    P,
                                    D,
                                    transpose=True,
                                    prepare_only=True,
                                    sem=gsems[s][buf],
                                    queue_num=q,
                                )
                                queues_used.add(q)
                            else:
                                gi = nc.gpsimd.dma_gather(
                                    gp[:],
                                    vtx[:],
                                    idx_sb[:, col + s * 8 : col + (s + 1) * 8],
                                    P,
                                    P,
                                    D,
                                    transpose=True,
                                    queue_num=q,
                                )
                                if use_semfix:
                                    gi.then_inc(gsems[s][buf], 16)
                            gps.append(gp)
                        for q in sorted(queues_used):
                            nc.gpsimd.trigger_dma(count=None, queue_num=q)
                        q01 = tmp.tile([P, JD, P], BF16, tag="q0")
                        if use_semfix >= 2:
                            nc.vector.wait_ge(gsems[0][buf], thr)
                            nc.vector.wait_ge(gsems[1][buf], thr)
                        nc.vector.tensor_add(q01[:], gps[0][:], gps[1][:])
                        q23 = tmp.tile([P, JD, P], BF16, tag="q1")
                        if use_semfix >= 2:
                            nc.vector.wait_ge(gsems[2][buf], thr)
                            nc.vector.wait_ge(gsems[3][buf], thr)
                        nc.vector.tensor_add(q23[:], gps[2][:], gps[3][:])
                        q45 = tmp.tile([P, JD, P], BF16, tag="q2")
                        if use_semfix >= 2:
                            nc.vector.wait_ge(gsems[4][buf], thr)
                            nc.vector.wait_ge(gsems[5][buf], thr)
                        nc.vector.tensor_add(q45[:], gps[4][:], gps[5][:])
                        r = tmp.tile([P, JD, P], BF16, tag="r")
                        nc.vector.tensor_add(r[:], q01[:], q23[:])
                        pl = plp.tile([P, JD, P], BF16, tag="p")
                        nc.vector.tensor_add(pl[:], r[:], q45[:])
                        _finish(lambda c: pl[:, c, :])
                    elif mode == "t3":
                        parts = []
                        for pi in range(3):
                            gp = gpool.tile([P, JD, 2 * P], BF16, tag=f"g{pi}")
                            nc.gpsimd.dma_gather(
                                gp[:],
                                vtx[:],
                                idx_sb[:, col + pi * 16 : col + (pi + 1) * 16],
                                2 * P,
                                2 * P,
                                D,
                                transpose=True,
                                queue_num=_swq(),
                            )
                            sp = tmp.tile([P, JD, P], BF16, tag=f"q{pi}")
                            nc.vector.tensor_add(
                                sp[:], gp[:, :, 0:P], gp[:, :, P : 2 * P]
                            )
                            parts.append(sp)
                        r = tmp.tile([P, JD, P], BF16, tag="r")
                        nc.vector.tensor_add(r[:], parts[0][:], parts[1][:])
                        pl = plp.tile([P, JD, P], BF16, tag="p")
                        nc.vector.tensor_add(pl[:], r[:], parts[2][:])
                        _finish(lambda c: pl[:, c, :])
                    elif mode == "r3":
                        parts = []
                        for pi in range(3):
                            gp = gpool.tile([P, 2, D], BF16, tag=f"g{pi}")
                            nc.gpsimd.dma_gather(
                                gp[:],
                                vtx[:],
                                idx_sb[:, col + pi * 16 : col + (pi + 1) * 16],
                                2 * P,
                                2 * P,
                                D,
                                queue_num=_swq(),
                            )
                            sp = tmp.tile([P, D], BF16, tag=f"q{pi}")
                            nc.vector.tensor_add(sp[:], gp[:, 0, :], gp[:, 1, :])
                            parts.append(sp)
                        ptp = ptps.tile([P, D], BF16)
                        for c in range(JD):
                            for pi in range(3):
                                nc.tensor.matmul(
                                    ptp[:, c * P : (c + 1) * P],
                                    lhsT=parts[pi][:, c * P : (c + 1) * P],
                                    rhs=ident[:],
                                    is_transpose=True,
                                    start=(pi == 0),
                                    stop=(pi == 2),
                                )
                        pl = plp.tile([P, D], BF16, tag="p")
                        nc.scalar.copy(pl[:], ptp[:])
                        _finish(lambda c: pl[:, c * P : (c + 1) * P])
                    else:
                        raise ValueError(mode)
    nc.finalize()
    return nc


def prep_inputs(vertex_feats, hex_to_vertex, W, b):
    """Host-side prep -> per-core in_maps."""
    vertex_feats = np.asarray(vertex_feats)
    hex_to_vertex = np.asarray(hex_to_vertex)
    W16 = np.ascontiguousarray(np.asarray(W).astype(ml_dtypes.bfloat16))
    b16 = np.asarray(b).astype(ml_dtypes.bfloat16).reshape(1, H)

    mask = hex_to_vertex >= 0
    safe = np.where(mask, hex_to_vertex, N).astype(np.int16)       # [T, 6]
    cntc = np.maximum(mask.sum(axis=1), 1).astype(np.float32)      # [T]
    inv = (1.0 / cntc).astype(np.float32)

    vtx_pads = []
    for bi in range(B):
        vp = np.zeros((N + 1, D), dtype=ml_dtypes.bfloat16)
        vp[:N] = vertex_feats[bi].astype(ml_dtypes.bfloat16)
        vtx_pads.append(vp)

    half_arrays = []
    for hh in range(2):
        sl = slice(hh * T_CORE, (hh + 1) * T_CORE)
        safe_h = safe[sl]                                          # [5000, 6]
        cnt_pad = np.ones(PADT, dtype=np.float32)
        cnt_pad[:T_CORE] = cntc[sl]
        inv_pad = np.ones(PADT, dtype=np.float32)
        inv_pad[:T_CORE] = inv[sl]
        # full tiles: flat[t, s*128 + p] = safe_h[t*128 + p, s]
        flat_full = (
            safe_h[: FULL * P]
            .reshape(FULL, P, S)
            .transpose(0, 2, 1)
            .reshape(FULL * S * P)
        )
        # tail: i = s*8 + h for s < 6, h < 8; rest -> zero row N
        flat_tail = np.full(TAIL_NI, N, dtype=np.int16)
        flat_tail[: S * LT_H] = safe_h[FULL * P : FULL * P + LT_H].T.reshape(
            S * LT_H
        )
        flat = np.concatenate([flat_full, flat_tail])
        # SWDGE idx wrap: column j, row p16 = flat[j*16 + p16]
        idx16 = flat.reshape(IDX_COLS, 16).T.astype(np.int16)
        idx_full = np.ascontiguousarray(np.tile(idx16, (8, 1)))    # 8 Q7 groups
        invc_arr = np.ascontiguousarray(inv_pad.reshape(TILES, P).T)  # [P, TILES]
        cnt_row = np.ascontiguousarray(
            cnt_pad.reshape(1, PADT).astype(ml_dtypes.bfloat16)
        )
        half_arrays.append((idx_full, invc_arr, cnt_row))

    in_maps = []
    for c in range(N_CORES):
        bi, hh = c // 2, c % 2
        idx_full, invc_arr, cnt_row = half_arrays[hh]
        in_maps.append(
            {
                "vtx": vtx_pads[bi],
                "wmat": W16,
                "brow": b16,
                "cnt": cnt_row,
                "idx": idx_full,
                "invc": invc_arr,
            }
        )
    return in_maps


def assemble_output(results):
    out = np.empty((B, T, H), dtype=np.float32)
    for c in range(N_CORES):
        bi, hh = c // 2, c % 2
        out[bi, hh * T_CORE : (hh + 1) * T_CORE] = (
            np.asarray(results[c]["out"][:T_CORE]).astype(np.float32)
        )
    return out


_CACHE = {}

# Host-side spot check: SWDGE gathers have a rare (~2%/run) silent-corruption
# mode on this HW where one DMA engine's rows of a ~2-tile window are stale
# (see the race notes in build_module). The kernel verifies 8 of the 512
# output columns against a host reference and re-runs the device program on
# mismatch. bf16 noise per element stays under ~0.03; corruption shows as
# O(0.5) errors on >=16 rows, so 8 columns at threshold 0.12 make a miss
# astronomically unlikely while false positives are ~impossible.
_CHECK_COLS = np.arange(0, H, H // 8)
_CHECK_THRESH = 0.12


def _spotcheck_ref(vertex_feats, hex_to_vertex, W, b):
    vf16 = np.asarray(vertex_feats).astype(ml_dtypes.bfloat16).astype(np.float32)
    h2v = np.asarray(hex_to_vertex)
    mask = h2v >= 0
    safe = np.where(mask, h2v, 0)
    cnt = np.maximum(mask.sum(1), 1).astype(np.float32)
    W8 = np.asarray(W)[:, _CHECK_COLS].astype(np.float32)
    b8 = np.asarray(b)[_CHECK_COLS].astype(np.float32)
    ref = np.empty((B, T, len(_CHECK_COLS)), np.float32)
    m = mask[:, :, None].astype(np.float32)
    for bi in range(B):
        pooled = (vf16[bi][safe] * m).sum(1) / cnt[:, None]     # (T, D)
        ref[bi] = pooled @ W8 + b8
    return ref


def kernel(vertex_feats, hex_to_vertex, W, b):
    include_bias = bool(np.any(np.asarray(b)))
    nc = _CACHE.get(include_bias)
    if nc is None:
        nc = build_module(include_bias=include_bias)
        _CACHE[include_bias] = nc
        _CACHE["nc"] = nc
    in_maps = prep_inputs(vertex_feats, hex_to_vertex, W, b)
    ref8 = _spotcheck_ref(vertex_feats, hex_to_vertex, W, b)
    out = None
    for attempt in range(6):
        res = run_bass_kernel_spmd(nc, in_maps, list(range(N_CORES)))
        out = assemble_output(res.results)
        bad = np.abs(out[:, :, _CHECK_COLS] - ref8) > _CHECK_THRESH
        if not bad.any():
            break
        nbad = int(bad.any(axis=2).sum())
        print(f"kernel: spot-check failed on {nbad} rows "
              f"(attempt {attempt}), re-running")
    return out


if __name__ == "__main__":
    rng = np.random.default_rng(0)
    vf = rng.standard_normal((B, N, D), dtype=np.float32)
    h2v = rng.integers(-1, N, size=(T, S), dtype=np.int64)
    W = (rng.standard_normal((D, H)) / np.sqrt(D)).astype(np.float32)
    b = np.zeros(H, dtype=np.float32)
    out = kernel(vertex_feats=vf, hex_to_vertex=h2v, W=W, b=b)
    print("out", out.shape, out.dtype, float(np.abs(out).max()))

